# revision 11
# baseline (speedup 1.0000x reference)
"""HardNet loss kernel for one TRN2 chip (8 NeuronCores), Bass/Tile.

Problem: x [16384, 256] f32, unit-L2 rows.  a = x[:8192] (anchors),
p = x[8192:] (positives).  dmat = sqrt((1 - a @ p.T + eps) * 2);
pos = diag(dmat); neg = min(row-min, col-min) of diag-masked dmat;
loss = mean(clip(1 - neg + pos, 0)).

Strategy (SPMD over 8 cores, no collectives):
  sqrt((1-d+eps)*2) is monotone-decreasing in d, so min-distance ==
  max-dot.  Each core owns a 1024-row slice of BOTH "query" sides:
    product 0: queries = its anchor slice,   keys = all positives
               -> row-max dot = per-anchor hard negative (row mins)
    product 1: queries = its positive slice, keys = all anchors
               -> row-max dot = per-positive hard negative (col mins)
  Both reduce along the matmul free dim (DVE reduce_max), so no
  cross-partition reductions and no allreduce are needed.  The diagonal
  must be excluded: host rolls each core's key columns by -1024*c so the
  core's own diagonal block always lands at columns [128*mt, 128*mt+128)
  of free-dim group 0, where a -4.0 eye tile is added before the max
  (dots are >= -1, so masked entries never win).  Host finishes with the
  8192-element min/sqrt/clip/mean (negligible).

Inputs are pre-transposed to [K, M]/[K, N] layout and cast to bf16 on
host (matmul contraction needs d on partitions; bf16 keeps dot error
~3e-3 which is far inside the loss tolerance, and pos is computed in
f32 on host exactly like the f32 reference).
"""

import numpy as np
import ml_dtypes

N_TOTAL = 16384
D = 256
CNT = N_TOTAL // 2          # 8192 anchor/positive pairs
NCORES = 8
SH = CNT // NCORES          # 1024 query rows per core per product
MTILES = SH // 128          # 8
NGRP = 4                    # free-dim groups of 2048 (4 PSUM banks)
GRP = CNT // NGRP           # 2048
NT = GRP // 512             # 4 matmuls of N=512 per group
MMN = 512                   # matmul moving free dim (ISA max per MM here)
EPS = 1e-6
MARGIN = 1.0
MASKVAL = -4.0              # added to diag dots; dots >= -1 so never max

_CACHE: dict = {}


def _build_program(reps: int = 1, mode: str = "full"):
    import concourse.tile as tile
    from concourse import bacc, mybir

    dt = mybir.dt
    nc = bacc.Bacc("TRN2", target_bir_lowering=False, debug=False,
                   num_devices=NCORES)

    rhs_d = nc.dram_tensor("rhs", [2, 128, 2 * CNT], dt.bfloat16,
                           kind="ExternalInput").ap()
    lhs_d = nc.dram_tensor("lhsT", [2, 128, 2 * SH], dt.bfloat16,
                           kind="ExternalInput").ap()
    eye_d = nc.dram_tensor("eyeneg", [128, 128], dt.float32,
                           kind="ExternalInput").ap()
    out_d = nc.dram_tensor("out", [128, 16], dt.float32,
                           kind="ExternalOutput").ap()

    with tile.TileContext(nc) as tc:
        with tc.tile_pool(name="big", bufs=1) as big, \
             tc.tile_pool(name="part", bufs=4) as partp, \
             tc.tile_pool(name="ps", bufs=2, space="PSUM") as psp:
            for rep in range(reps):
                lhs_sb = big.tile([128, 2 * 2 * SH], dt.bfloat16, tag="lhs")
                eye_sb = big.tile([128, 128], dt.float32, tag="eye")
                outsb = big.tile([128, 16], dt.float32, tag="outsb")
                if mode != "null":
                    nc.sync.dma_start(lhs_sb[:, 0:2 * SH], lhs_d[0])
                    nc.sync.dma_start(lhs_sb[:, 2 * SH:4 * SH], lhs_d[1])
                    nc.sync.dma_start(eye_sb[:], eye_d[:])

                # rhs chunks: [kc][g][grp] -> [128, GRP] bf16
                rhs_sb = {}
                for g in (0, 1):
                    for grp in range(NGRP):
                        for kc in (0, 1):
                            t = big.tile([128, GRP], dt.bfloat16,
                                         tag=f"rhs_{kc}_{g}_{grp}",
                                         name=f"rhs_{kc}_{g}_{grp}")
                            col = g * CNT + grp * GRP
                            if mode != "null":
                                nc.sync.dma_start(t[:], rhs_d[kc, :, col:col + GRP])
                            rhs_sb[(kc, g, grp)] = t

                if mode in ("dmaonly", "null"):
                    ps = psp.tile([128, GRP], dt.float32, tag="ps", name="ps_x")
                    nc.tensor.matmul(ps[:, 0:512], lhs_sb[:, 0:128],
                                     rhs_sb[(0, 0, 0)][:, 0:512],
                                     start=True, stop=True)
                    nc.vector.reduce_max(outsb[:, 0:1], ps[:, 0:4],
                                         axis=mybir.AxisListType.X)
                    nc.sync.dma_start(out_d[:], outsb[:])
                    continue

                for g in (0, 1):
                    for mt in range(MTILES):
                        part = partp.tile([128, NGRP], dt.float32, tag="part",
                                          name=f"part_{g}_{mt}")
                        for grp in range(NGRP):
                            ps = psp.tile([128, GRP], dt.float32, tag="ps",
                                          name=f"ps_{g}_{mt}_{grp}")
                            if mode != "nomm":
                                for kc in (0, 1):
                                    w = lhs_sb[:, kc * 2 * SH + g * SH + mt * 128:
                                               kc * 2 * SH + g * SH + (mt + 1) * 128]
                                    for nt in range(GRP // MMN):
                                        nc.tensor.matmul(
                                            ps[:, nt * MMN:(nt + 1) * MMN],
                                            w,
                                            rhs_sb[(kc, g, grp)][:, nt * MMN:(nt + 1) * MMN],
                                            start=(kc == 0),
                                            stop=(kc == 1),
                                        )
                            else:
                                nc.tensor.matmul(
                                    ps[:, 0:512],
                                    lhs_sb[:, 0:128],
                                    rhs_sb[(0, g, grp)][:, 0:512],
                                    start=True, stop=True,
                                )
                            if mode == "tinyred":
                                nc.vector.reduce_max(
                                    part[:, grp:grp + 1], ps[:, 0:4],
                                    axis=mybir.AxisListType.X,
                                )
                                continue
                            if mode == "nored":
                                continue
                            if grp == 0:
                                dcol = mt * 128
                                nc.vector.tensor_add(
                                    ps[:, dcol:dcol + 128],
                                    ps[:, dcol:dcol + 128],
                                    eye_sb[:],
                                )
                            nc.vector.reduce_max(
                                part[:, grp:grp + 1], ps[:],
                                axis=mybir.AxisListType.X,
                            )
                        if mode == "nored":
                            # keep a tiny dep so outsb/part still flow
                            nc.vector.reduce_max(
                                part[:, 0:1], ps[:, 0:4],
                                axis=mybir.AxisListType.X,
                            )
                        nc.vector.reduce_max(
                            outsb[:, g * MTILES + mt:g * MTILES + mt + 1],
                            part[:],
                            axis=mybir.AxisListType.X,
                        )
                nc.sync.dma_start(out_d[:], outsb[:])

    nc.compile()
    return nc


def _prep_inputs(x: np.ndarray):
    """Build per-core in_maps from full x."""
    x = np.ascontiguousarray(x, dtype=np.float32)
    xT16 = np.ascontiguousarray(x.T).astype(ml_dtypes.bfloat16)  # [256, 16384]
    X3 = xT16.reshape(2, 128, N_TOTAL)
    aT = X3[:, :, :CNT]     # [2, 128, 8192]
    pT = X3[:, :, CNT:]
    keys = (pT, aT)         # product 0 keys = positives, product 1 = anchors
    queries = (aT, pT)

    eye = (np.eye(128, dtype=np.float32) * MASKVAL).astype(np.float32)

    in_maps = []
    for c in range(NCORES):
        r = SH * c
        rhs = np.empty((2, 128, 2 * CNT), dtype=ml_dtypes.bfloat16)
        lhsT = np.empty((2, 128, 2 * SH), dtype=ml_dtypes.bfloat16)
        for g in (0, 1):
            K = keys[g]
            if r == 0:
                rhs[:, :, g * CNT:(g + 1) * CNT] = K
            else:
                rhs[:, :, g * CNT:g * CNT + (CNT - r)] = K[:, :, r:]
                rhs[:, :, g * CNT + (CNT - r):(g + 1) * CNT] = K[:, :, :r]
            lhsT[:, :, g * SH:(g + 1) * SH] = queries[g][:, :, r:r + SH]
        in_maps.append({"rhs": rhs, "lhsT": lhsT, "eyeneg": eye})
    return in_maps


def _finish(x: np.ndarray, results) -> np.ndarray:
    a = x[:CNT].astype(np.float32)
    p = x[CNT:].astype(np.float32)
    posdot = np.sum(a * p, axis=1, dtype=np.float32)

    maxA = np.concatenate(
        [results[c]["out"][:, 0:MTILES].T.reshape(-1) for c in range(NCORES)])
    maxB = np.concatenate(
        [results[c]["out"][:, MTILES:2 * MTILES].T.reshape(-1) for c in range(NCORES)])
    negdot = np.maximum(maxA, maxB).astype(np.float32)

    neg = np.sqrt((1.0 - negdot + EPS) * 2.0)
    pos = np.sqrt((1.0 - posdot + EPS) * 2.0)
    loss = np.mean(np.clip(MARGIN - neg + pos, 0.0, None))
    return np.asarray(loss, dtype=np.float32)


def kernel(x: np.ndarray) -> np.ndarray:
    from concourse import bass_utils

    if "nc" not in _CACHE:
        _CACHE["nc"] = _build_program()
    nc = _CACHE["nc"]

    in_maps = _prep_inputs(x)
    res = bass_utils.run_bass_kernel_spmd(nc, in_maps,
                                          core_ids=list(range(NCORES)))
    return _finish(x, res.results)
